# revision 28
# baseline (speedup 1.0000x reference)
"""Causal self-attention (B=4, T=2048, C=1024, NH=16) on 8 TRN2 NeuronCores.

Sharding (per spec hint): tensor-parallel over heads x data-parallel over batch.
Core i handles batch b = i//2 and head-group g = i%2 (8 heads each).
  - c_attn column-parallel: each core computes q,k,v for its 8 heads.
  - attention: fully local per core (its heads, its batch element).
  - c_proj row-parallel: each core computes a bf16 partial (yT@wp + 0.5*bp)
    per 512-token block; a 2-core ReduceScatter over pairs
    [[0,1],[2,3],[4,5],[6,7]] sums each block's partials as soon as they
    exist; cheap bf16 DRAM copies move each 256-row shard to the output,
    overlapped except for the final block. Host casts to fp32 + concatenates.

Device algorithm (per core), all matmuls bf16 with fp32 PSUM accumulation:
  xT (C,T) staged transposed by host; input DMAs are split per weight/x chunk
  and ordered by first use, so the first qkv matmul starts after ~0.8MB
  instead of the full 8MB.
  qT = wq^T @ xT, kT = wk^T @ xT   (feature-major, 4 chunks of 128)
  v  = x @ wv                      (token-major) + ones column per head
  per head pair (2fc, 2fc+1), per q-block Q (512 wide):
    s^T[kchunk] = kT_h^T @ qT_h    (K=64 contraction, row-tiled pair -> concurrent)
    p = exp(0.125 * s^T)  (ScalarE, bf16 out); causal-zeroed on GpSimd for
        diagonal chunks; fully-masked chunks skipped entirely. Diagonal
        chunks are triangle-trimmed: QK/exp/select/AV only touch query
        columns >= 128*(kc-4Q), cutting ~12% of PE work and ~15% of exp.
    o^T[65,512] += v_aug_h^T @ p   (v_aug has a ones column -> row 64 = softmax
        denominators, fused into the same matmul)
    yT_h = o^T[0:64] * (1/o^T[64])  (DVE recip + GpSimd partition_broadcast,
        no DRAM bounce)
  c_proj token-blocks ride the exp-bound slack of later attention blocks;
  the tail is only: proj(block 3) -> RS(3) -> done.
"""

import sys

if "/opt/trn_rl_repo" not in sys.path:
    sys.path.insert(0, "/opt/trn_rl_repo")

import numpy as np
import ml_dtypes

import concourse.bass as bass
import concourse.bacc as bacc
import concourse.mybir as mybir
import concourse.tile as tile
from concourse.bass import ts, ds
from concourse.bass_utils import run_bass_kernel_spmd

BF16 = ml_dtypes.bfloat16
N_CORES = 8
B, T, C = 4, 2048, 1024
NH, HS = 16, 64
H_LOC = NH // 2        # heads per core
F = H_LOC * HS         # 512 local qkv features
NFC = F // 128         # 4 feature chunks (one head pair each)
NKC = T // 128         # 16 key chunks
NQ = T // 512          # 4 query blocks
NCOL = C // 512        # 2 output column blocks
KO = C // 128          # 8 contraction chunks for the qkv projections
REPLICA_GROUPS = [[0, 1], [2, 3], [4, 5], [6, 7]]

FP32 = mybir.dt.float32
BF = mybir.dt.bfloat16


def _build_nc():
    # Bacc (not plain Bass): its compile() pipeline runs
    # generate_event_semaphores, which splits sync waits so no instruction
    # carries more than the hardware allows (walrus rejects >1 otherwise).
    nc = bacc.Bacc(None, target_bir_lowering=False, num_devices=N_CORES)

    xT = nc.dram_tensor("xT", [C, T], BF, kind="ExternalInput")
    wq = nc.dram_tensor("wq", [C, F], BF, kind="ExternalInput")
    wk = nc.dram_tensor("wk", [C, F], BF, kind="ExternalInput")
    wv = nc.dram_tensor("wv", [C, F], BF, kind="ExternalInput")
    bq = nc.dram_tensor("bq", [F], FP32, kind="ExternalInput")
    bk = nc.dram_tensor("bk", [F], FP32, kind="ExternalInput")
    bv = nc.dram_tensor("bv", [F], FP32, kind="ExternalInput")
    wp = nc.dram_tensor("wp", [F, C], BF, kind="ExternalInput")
    bp = nc.dram_tensor("bp", [C], FP32, kind="ExternalInput")
    out = nc.dram_tensor("out", [T // 2, C], BF, kind="ExternalOutput")

    with tile.TileContext(nc) as tc:
        _body(tc, xT, wq, wk, wv, bq, bk, bv, wp, bp, out)
    nc.compile()
    return nc


def _body(tc, xT, wq, wk, wv, bq, bk, bv, wp, bp, out):
    nc = tc.nc
    import contextlib

    ctx = contextlib.ExitStack()
    with ctx:
        wpool = ctx.enter_context(tc.tile_pool(name="weights", bufs=1))
        apool = ctx.enter_context(tc.tile_pool(name="acts", bufs=1))
        ppool = ctx.enter_context(tc.tile_pool(name="ptiles", bufs=3))
        npool = ctx.enter_context(tc.tile_pool(name="norm", bufs=2))
        outp = ctx.enter_context(tc.tile_pool(name="outsb", bufs=2))
        # PSUM budget (8 banks): sAB [128,1024] x3 bufs = 6, oA/oB 1 bank each = 2
        ps_s = ctx.enter_context(tc.tile_pool(name="ps_s", bufs=3, space="PSUM"))
        ps_o = ctx.enter_context(tc.tile_pool(name="ps_o", bufs=1, space="PSUM"))
        dpool = ctx.enter_context(tc.tile_pool(name="dram", bufs=1, space="DRAM"))

        # ---- stage inputs into SBUF, split per chunk and ordered by first
        # use: the prefix consumes (wq|wk chunk fc=0) + all 8 x chunks of
        # tokens 0:1024 first, so matmuls start after ~0.8MB of DMA.
        wq_sb = wpool.tile([128, KO, F], BF)
        wk_sb = wpool.tile([128, KO, F], BF)
        wq_r = wq.rearrange("(ko p) f -> p ko f", p=128)
        wk_r = wk.rearrange("(ko p) f -> p ko f", p=128)
        nc.sync.dma_start(out=wq_sb[:, :, ts(0, 128)], in_=wq_r[:, :, ts(0, 128)])
        nc.sync.dma_start(out=wk_sb[:, :, ts(0, 128)], in_=wk_r[:, :, ts(0, 128)])
        x_sb = wpool.tile([128, KO, T], BF)
        for kc in range(KO):
            nc.sync.dma_start(
                out=x_sb[:, kc, 0 : T // 2],
                in_=xT.ap()[ds(128 * kc, 128), 0 : T // 2],
            )
        # biases ride behind the first x chunks (first use: end of unit fc=0)
        bq_sb = wpool.tile([128, NFC], FP32)
        nc.sync.dma_start(out=bq_sb, in_=bq.rearrange("(fo p) -> p fo", p=128))
        bk_sb = wpool.tile([128, NFC], FP32)
        nc.sync.dma_start(out=bk_sb, in_=bk.rearrange("(fo p) -> p fo", p=128))
        for fc in range(1, NFC):
            nc.sync.dma_start(
                out=wq_sb[:, :, ts(fc, 128)], in_=wq_r[:, :, ts(fc, 128)]
            )
            nc.sync.dma_start(
                out=wk_sb[:, :, ts(fc, 128)], in_=wk_r[:, :, ts(fc, 128)]
            )
        # broadcast biases across partitions (for token-major layouts)
        bv_bc = wpool.tile([128, F], FP32)
        nc.sync.dma_start(
            out=bv_bc,
            in_=bass.AP(tensor=bv.ap().tensor, offset=0, ap=[[0, 128], [1, F]]),
        )
        wv_sb = wpool.tile([128, KO, F], BF)
        nc.sync.dma_start(out=wv_sb, in_=wv.rearrange("(ko p) f -> p ko f", p=128))
        xT_r = xT.rearrange("(ko p) t -> p ko t", p=128)
        nc.sync.dma_start(out=x_sb[:, :, T // 2 :], in_=xT_r[:, :, T // 2 :])
        wp_sb = wpool.tile([128, NFC, C], BF)
        nc.sync.dma_start(out=wp_sb, in_=wp.rearrange("(ko p) n -> p ko n", p=128))
        bp_bc = wpool.tile([128, C], FP32)
        nc.sync.dma_start(
            out=bp_bc,
            in_=bass.AP(tensor=bp.ap().tensor, offset=0, ap=[[0, 128], [1, C]]),
        )

        # ---- persistent activations ----
        qT_sb = apool.tile([128, NFC, T], BF)   # q, feature-major
        kT_sb = apool.tile([128, NFC, T], BF)   # k, feature-major
        # v token-major, 66-stride per head: cols 0:64 = v, col 64 = ones
        v_sb = apool.tile([128, NKC, H_LOC, 66], BF)
        nc.vector.memset(v_sb[:, :, :, 64:65], 1.0)
        yT_sb = apool.tile([128, NFC, T], BF)   # attention out, feature-major

        partial = dpool.tile([T, C], BF)        # c_proj partial (pre-reduce)
        # per-Q-block ReduceScatter halves: core keeps [256,1024] per block
        rs_outs = [dpool.tile([256, C], BF, name=f"rs_out{q}") for q in range(NQ)]

        # ---- qkv projection units (emitted piecemeal: half up front, the
        # rest interleaved into the exp-bound attention phase as PE filler) --
        def qk_unit(w_sb, b_sb, dst, fc, tq2):
            # one 1024-token span of q^T or k^T for head-pair chunk fc
            ps = ps_s.tile([128, 1024], FP32, tag="sAB")
            for kc in range(KO):
                for half in range(2):
                    nc.tensor.matmul(
                        ps[:, ts(half, 512)],
                        lhsT=w_sb[:, kc, ts(fc, 128)],
                        rhs=x_sb[:, kc, ds(tq2 * 1024 + half * 512, 512)],
                        start=(kc == 0),
                        stop=(kc == KO - 1),
                    )
            nc.scalar.activation(
                out=dst[:, fc, ts(tq2, 1024)],
                in_=ps,
                func=mybir.ActivationFunctionType.Identity,
                bias=b_sb[:, fc : fc + 1],
                scale=1.0,
            )

        def v_unit(tc_i):
            ps = ps_s.tile([128, 1024], FP32, tag="sAB")
            for kc in range(KO):
                nc.tensor.matmul(
                    ps[:, 0:512],
                    lhsT=x_sb[:, kc, ts(tc_i, 128)],
                    rhs=wv_sb[:, kc, :],
                    start=(kc == 0),
                    stop=(kc == KO - 1),
                )
            nc.vector.tensor_add(
                out=v_sb[:, tc_i, :, 0:64],
                in0=ps[:, 0:512].rearrange("p (h f) -> p h f", h=H_LOC),
                in1=bv_bc.rearrange("p (h f) -> p h f", h=H_LOC),
            )

        # prefix: everything attention blocks 0-1 need
        for fc in range(NFC):
            qk_unit(wq_sb, bq_sb, qT_sb, fc, 0)
            qk_unit(wk_sb, bk_sb, kT_sb, fc, 0)
        for tc_i in range(8):
            v_unit(tc_i)

        def proj_tb(Q, tb):
            # bf16 partial (+0.5*bp, host-folded) for token row-block Q*4+tb
            trow = Q * 4 + tb
            ps = ps_s.tile([128, 1024], FP32, tag="sAB")
            # fc outer / ncol inner: consecutive matmuls alternate the two
            # PSUM banks of ps, keeping back-to-back issue fast
            for fc in range(NFC):
                for ncol in range(NCOL):
                    nc.tensor.matmul(
                        ps[:, ts(ncol, 512)],
                        lhsT=yT_sb[:, fc, ts(trow, 128)],
                        rhs=wp_sb[:, fc, ts(ncol, 512)],
                        start=(fc == 0),
                        stop=(fc == NFC - 1),
                    )
            p_sb = outp.tile([128, 1024], BF, tag="psb")
            nc.vector.tensor_add(out=p_sb, in0=ps, in1=bp_bc)
            nc.sync.dma_start(out=partial[ds(trow * 128, 128), :], in_=p_sb)

        def rs_block(Q):
            # reduce this 512-token block across the batch pair while later
            # blocks still compute; each core keeps 256 of the 512 rows.
            nc.gpsimd.collective_compute(
                "ReduceScatter",
                mybir.AluOpType.add,
                replica_groups=REPLICA_GROUPS,
                ins=[partial[ds(Q * 512, 512), :]],
                outs=[rs_outs[Q][:]],
            )

        def copy_out(Q):
            # bf16 shard -> output rows; emitted well after rs_block(Q) so
            # the sync-queue sem wait is (nearly) clear when it issues.
            nc.sync.dma_start(out=out.ap()[ds(Q * 256, 256), :], in_=rs_outs[Q][:])

        # filler units with emission deadlines:
        #   qT second halves + v 8..11 -> before attention(2) reads them
        #   kT second halves + v 12..15 -> before attention(3)
        # proj(Q) rides block Q+1's slack; rs(Q) fires as soon as the gpsimd
        # queue drains past the preceding normalize broadcasts.
        slot_work = {
            0: {fc: [lambda fc=fc: qk_unit(wq_sb, bq_sb, qT_sb, fc, 1),
                     lambda i=fc + 8: v_unit(i)] for fc in range(NFC)},
            1: {fc: [lambda fc=fc: qk_unit(wk_sb, bk_sb, kT_sb, fc, 1),
                     lambda i=fc + 12: v_unit(i),
                     lambda tb=fc: proj_tb(0, tb)] for fc in range(NFC)},
            2: {0: [lambda: proj_tb(1, 0)],
                1: [lambda: proj_tb(1, 1)],
                2: [lambda: proj_tb(1, 2)],
                3: [lambda: proj_tb(1, 3)]},
            3: {0: [lambda: proj_tb(2, 0), lambda: proj_tb(2, 1)],
                1: [lambda: proj_tb(2, 2), lambda: proj_tb(2, 3)],
                2: [lambda: rs_block(2)]},
        }
        after_block = {1: [lambda: rs_block(0)], 2: [lambda: rs_block(1)]}

        # ---- attention per q-block ----
        def attention_block(Q, slots, early=False):
            # early=True: emit the slot fillers between the QK/exp stream and
            # the AV drain -- on short (all-diagonal) blocks the in-order PE
            # queue otherwise idles behind AVs waiting for exp+select.
            nkc = 4 * Q + 4  # causal: only key chunks 0 .. 4Q+3 contribute
            LAG = 3  # AV matmuls trail the QK/exp pipeline by this many chunks
            for fc in range(NFC):  # head pair (2fc, 2fc+1)
                oA = ps_o.tile([65, 512], FP32, tag="oA")
                oB = ps_o.tile([65, 512], FP32, tag="oB")
                pbuf = {}

                def emit_av(kc, oA=oA, oB=oB, nkc=nkc, fc=fc, Q=Q):
                    pAB = pbuf.pop(kc)
                    q0 = max(0, 128 * (kc - 4 * Q))
                    w = 512 - q0
                    nc.tensor.matmul(
                        oA[:, ds(q0, w)],
                        lhsT=v_sb[:, kc, 2 * fc, 0:65],
                        rhs=pAB[:, ds(q0, w)],
                        start=(kc == 0),
                        stop=(kc == nkc - 1),
                    )
                    nc.tensor.matmul(
                        oB[:, ds(q0, w)],
                        lhsT=v_sb[:, kc, 2 * fc + 1, 0:65],
                        rhs=pAB[:, ds(512 + q0, w)],
                        start=(kc == 0),
                        stop=(kc == nkc - 1),
                    )

                for kc in range(nkc):
                    # heads A and B share one 2-bank psum tile: A in cols
                    # 0:512 (array rows 0:64), B in 512:1024 (rows 64:128);
                    # the row-tiled pair runs concurrently on the PE.
                    # Diagonal chunks only touch query columns >= q0.
                    q0 = max(0, 128 * (kc - 4 * Q))
                    w = 512 - q0
                    sAB = ps_s.tile([128, 1024], FP32, tag="sAB")
                    nc.tensor.matmul(
                        sAB[:, ds(q0, w)],
                        lhsT=kT_sb[0:64, fc, ts(kc, 128)],
                        rhs=qT_sb[0:64, fc, ds(512 * Q + q0, w)],
                        start=True,
                        stop=True,
                        tile_position=(0, 0),
                    )
                    nc.tensor.matmul(
                        sAB[:, ds(512 + q0, w)],
                        lhsT=kT_sb[64:128, fc, ts(kc, 128)],
                        rhs=qT_sb[64:128, fc, ds(512 * Q + q0, w)],
                        start=True,
                        stop=True,
                        tile_position=(64, 0),
                    )
                    pAB = ppool.tile([128, 1024], BF, tag="pAB", bufs=5)
                    nc.scalar.activation(
                        out=pAB.rearrange("p (h q) -> p h q", h=2)[:, :, q0:],
                        in_=sAB.rearrange("p (h q) -> p h q", h=2)[:, :, q0:],
                        func=mybir.ActivationFunctionType.Exp,
                        scale=0.125,
                    )
                    if kc >= 4 * Q:
                        # crosses the causal boundary: zero exp of masked
                        # scores (k_global > q_global) for both head halves
                        nc.gpsimd.affine_select(
                            out=pAB.rearrange("p (h q) -> p h q", h=2)[:, :, q0:],
                            in_=pAB.rearrange("p (h q) -> p h q", h=2)[:, :, q0:],
                            compare_op=mybir.AluOpType.is_ge,
                            fill=0.0,
                            base=0,
                            channel_multiplier=-1,
                            pattern=[[0, 2], [1, w]],
                        )
                    pbuf[kc] = pAB
                    if kc >= LAG:
                        emit_av(kc - LAG)
                if early:
                    for work in slots.get(fc, ()):
                        work()
                for kc in range(max(0, nkc - LAG), nkc):
                    emit_av(kc)
                # normalize: yT_h = oT[0:64] * (1 / oT[64]).
                oA_sb = npool.tile([65, 512], FP32, tag="oAsb")
                oB_sb = npool.tile([65, 512], FP32, tag="oBsb")
                nc.vector.tensor_copy(out=oA_sb, in_=oA)
                nc.vector.tensor_copy(out=oB_sb, in_=oB)
                # custom-DVE reciprocal_approx_fast mishandles inputs at a
                # nonzero partition base -- stage row 64 down to partition 0
                rzA = npool.tile([1, 512], FP32, tag="rzA", bufs=1)
                rzB = npool.tile([1, 512], FP32, tag="rzB", bufs=1)
                nc.vector.tensor_copy(out=rzA, in_=oA_sb[64:65, :])
                nc.vector.tensor_copy(out=rzB, in_=oB_sb[64:65, :])
                rA = npool.tile([1, 512], FP32, tag="rA", bufs=1)
                rB = npool.tile([1, 512], FP32, tag="rB", bufs=1)
                nc.vector.reciprocal_approx_fast(out=rA, in_=rzA)
                nc.vector.reciprocal_approx_fast(out=rB, in_=rzB)
                # partition-broadcast on GpSimd (no DRAM bounce)
                bcA = npool.tile([64, 512], FP32, tag="bcA", bufs=1)
                bcB = npool.tile([64, 512], FP32, tag="bcB", bufs=1)
                nc.gpsimd.partition_broadcast(bcA, rA)
                nc.gpsimd.partition_broadcast(bcB, rB)
                # head A lives on partitions 0:64 of chunk fc
                nc.vector.tensor_mul(
                    out=yT_sb[0:64, fc, ts(Q, 512)], in0=oA_sb[0:64, :], in1=bcA
                )
                # head B must land on partitions 64:128 -> stage + DMA shift
                yB = npool.tile([64, 512], BF, tag="yB")
                nc.vector.tensor_mul(out=yB, in0=oB_sb[0:64, :], in1=bcB)
                nc.sync.dma_start(out=yT_sb[64:128, fc, ts(Q, 512)], in_=yB)

                if not early:
                    for work in slots.get(fc, ()):
                        work()

        for Q in range(NQ):
            attention_block(Q, slot_work.get(Q, {}), early=(Q <= 1))
            for work in after_block.get(Q, ()):
                work()
        # tail: only the last block's proj + reduce remain; all output copies
        # sit here so their collective waits never head-block mid-kernel DMAs
        for tb in range(4):
            proj_tb(NQ - 1, tb)
        copy_out(0)
        copy_out(1)
        rs_block(NQ - 1)
        copy_out(2)
        copy_out(NQ - 1)


_NC_CACHE = None


def _get_nc():
    global _NC_CACHE
    if _NC_CACHE is None:
        _NC_CACHE = _build_nc()
    return _NC_CACHE


def kernel(x, w_attn, b_attn, w_proj, b_proj):
    x = np.asarray(x)
    w_attn = np.asarray(w_attn)
    b_attn = np.asarray(b_attn)
    w_proj = np.asarray(w_proj)
    b_proj = np.asarray(b_proj)

    nc = _get_nc()

    in_maps = []
    for i in range(N_CORES):
        b, g = i // 2, i % 2
        in_maps.append(
            {
                "xT": np.ascontiguousarray(x[b].T).astype(BF16),
                "wq": np.ascontiguousarray(w_attn[:, g * F : (g + 1) * F]).astype(BF16),
                "wk": np.ascontiguousarray(
                    w_attn[:, C + g * F : C + (g + 1) * F]
                ).astype(BF16),
                "wv": np.ascontiguousarray(
                    w_attn[:, 2 * C + g * F : 2 * C + (g + 1) * F]
                ).astype(BF16),
                "bq": np.ascontiguousarray(b_attn[g * F : (g + 1) * F]).astype(
                    np.float32
                ),
                "bk": np.ascontiguousarray(b_attn[C + g * F : C + (g + 1) * F]).astype(
                    np.float32
                ),
                "bv": np.ascontiguousarray(
                    b_attn[2 * C + g * F : 2 * C + (g + 1) * F]
                ).astype(np.float32),
                "wp": np.ascontiguousarray(w_proj[g * F : (g + 1) * F, :]).astype(BF16),
                "bp": (b_proj * 0.5).astype(np.float32),
            }
        )

    global _last_in_maps
    _last_in_maps = in_maps  # stashed for external profiling harnesses
    res = run_bass_kernel_spmd(nc, in_maps, core_ids=list(range(N_CORES)))

    # Each core's "out" holds NQ blocks of 256 rows: block Q is the core's
    # ReduceScatter half of token rows [Q*512, (Q+1)*512) -- rank 0 (even
    # core) the first 256, rank 1 (odd core) the last 256.
    out = np.empty((B, T, C), dtype=np.float32)
    for b in range(B):
        even = res.results[2 * b]["out"].astype(np.float32).reshape(NQ, 256, C)
        odd = res.results[2 * b + 1]["out"].astype(np.float32).reshape(NQ, 256, C)
        blocks = out[b].reshape(NQ, 2, 256, C)
        blocks[:, 0] = even
        blocks[:, 1] = odd
    return out


# revision 37
# speedup vs baseline: 1.0004x; 1.0004x over previous
"""Causal self-attention (B=4, T=2048, C=1024, NH=16) on 8 TRN2 NeuronCores.

Sharding (per spec hint): tensor-parallel over heads x data-parallel over batch.
Core i handles batch b = i//2 and head-group g = i%2 (8 heads each).
  - c_attn column-parallel: each core computes q,k,v for its 8 heads.
  - attention: fully local per core (its heads, its batch element).
  - c_proj row-parallel: each core computes a bf16 partial (yT@wp + 0.5*bp)
    per 512-token block; a 2-core ReduceScatter over pairs
    [[0,1],[2,3],[4,5],[6,7]] sums each block's partials as soon as they
    exist; cheap bf16 DRAM copies move each 256-row shard to the output,
    overlapped except for the final block. Host casts to fp32 + concatenates.

Device algorithm (per core), all matmuls bf16 with fp32 PSUM accumulation:
  xT (C,T) staged transposed by host; input DMAs are split per weight/x chunk
  and ordered by first use, so the first qkv matmul starts after ~0.8MB
  instead of the full 8MB.
  qT = wq^T @ xT, kT = wk^T @ xT   (feature-major, 4 chunks of 128)
  v  = x @ wv                      (token-major) + ones column per head
  per head pair (2fc, 2fc+1), per q-block Q (512 wide):
    s^T[kchunk] = kT_h^T @ qT_h    (K=64 contraction, row-tiled pair -> concurrent)
    p = exp(0.125 * s^T)  (ScalarE, bf16 out); causal-zeroed on GpSimd for
        diagonal chunks; fully-masked chunks skipped entirely. Diagonal
        chunks are triangle-trimmed: QK/exp/select/AV only touch query
        columns >= 128*(kc-4Q), cutting ~12% of PE work and ~15% of exp.
    o^T[65,512] += v_aug_h^T @ p   (v_aug has a ones column -> row 64 = softmax
        denominators, fused into the same matmul)
    yT_h = o^T[0:64] * (1/o^T[64])  (DVE recip + GpSimd partition_broadcast,
        no DRAM bounce)
  c_proj token-blocks ride the exp-bound slack of later attention blocks;
  the tail is only: proj(block 3) -> RS(3) -> done.
"""

import sys

if "/opt/trn_rl_repo" not in sys.path:
    sys.path.insert(0, "/opt/trn_rl_repo")

import numpy as np
import ml_dtypes

import concourse.bass as bass
import concourse.bacc as bacc
import concourse.mybir as mybir
import concourse.tile as tile
from concourse.bass import ts, ds
from concourse.bass_utils import run_bass_kernel_spmd

BF16 = ml_dtypes.bfloat16
N_CORES = 8
B, T, C = 4, 2048, 1024
NH, HS = 16, 64
H_LOC = NH // 2        # heads per core
F = H_LOC * HS         # 512 local qkv features
NFC = F // 128         # 4 feature chunks (one head pair each)
NKC = T // 128         # 16 key chunks
NQ = T // 512          # 4 query blocks
NCOL = C // 512        # 2 output column blocks
KO = C // 128          # 8 contraction chunks for the qkv projections
REPLICA_GROUPS = [[0, 1], [2, 3], [4, 5], [6, 7]]

FP32 = mybir.dt.float32
BF = mybir.dt.bfloat16


def _build_nc():
    # Bacc (not plain Bass): its compile() pipeline runs
    # generate_event_semaphores, which splits sync waits so no instruction
    # carries more than the hardware allows (walrus rejects >1 otherwise).
    nc = bacc.Bacc(None, target_bir_lowering=False, num_devices=N_CORES)

    xT = nc.dram_tensor("xT", [C, T], BF, kind="ExternalInput")
    wq = nc.dram_tensor("wq", [C, F], BF, kind="ExternalInput")
    wk = nc.dram_tensor("wk", [C, F], BF, kind="ExternalInput")
    wv = nc.dram_tensor("wv", [C, F], BF, kind="ExternalInput")
    bq = nc.dram_tensor("bq", [F], FP32, kind="ExternalInput")
    bk = nc.dram_tensor("bk", [F], FP32, kind="ExternalInput")
    bv = nc.dram_tensor("bv", [F], FP32, kind="ExternalInput")
    wp = nc.dram_tensor("wp", [F, C], BF, kind="ExternalInput")
    bp = nc.dram_tensor("bp", [C], FP32, kind="ExternalInput")
    out = nc.dram_tensor("out", [T // 2, C], BF, kind="ExternalOutput")

    with tile.TileContext(nc) as tc:
        _body(tc, xT, wq, wk, wv, bq, bk, bv, wp, bp, out)
    nc.compile()
    return nc


def _body(tc, xT, wq, wk, wv, bq, bk, bv, wp, bp, out):
    nc = tc.nc
    import contextlib

    ctx = contextlib.ExitStack()
    with ctx:
        wpool = ctx.enter_context(tc.tile_pool(name="weights", bufs=1))
        apool = ctx.enter_context(tc.tile_pool(name="acts", bufs=1))
        ppool = ctx.enter_context(tc.tile_pool(name="ptiles", bufs=3))
        npool = ctx.enter_context(tc.tile_pool(name="norm", bufs=2))
        outp = ctx.enter_context(tc.tile_pool(name="outsb", bufs=2))
        # PSUM budget (8 banks): sAB [128,1024] x3 bufs = 6, oA/oB 1 bank each = 2
        ps_s = ctx.enter_context(tc.tile_pool(name="ps_s", bufs=3, space="PSUM"))
        ps_o = ctx.enter_context(tc.tile_pool(name="ps_o", bufs=1, space="PSUM"))
        dpool = ctx.enter_context(tc.tile_pool(name="dram", bufs=1, space="DRAM"))

        # ---- stage inputs into SBUF, split per chunk and ordered by first
        # use: the prefix consumes (wq|wk chunk fc=0) + all 8 x chunks of
        # tokens 0:1024 first, so matmuls start after ~0.8MB of DMA.
        bq_sb = wpool.tile([128, NFC], FP32)
        nc.sync.dma_start(out=bq_sb, in_=bq.rearrange("(fo p) -> p fo", p=128))
        bk_sb = wpool.tile([128, NFC], FP32)
        nc.sync.dma_start(out=bk_sb, in_=bk.rearrange("(fo p) -> p fo", p=128))

        wq_sb = wpool.tile([128, KO, F], BF)
        wk_sb = wpool.tile([128, KO, F], BF)
        wq_r = wq.rearrange("(ko p) f -> p ko f", p=128)
        wk_r = wk.rearrange("(ko p) f -> p ko f", p=128)
        nc.sync.dma_start(out=wq_sb[:, :, ts(0, 128)], in_=wq_r[:, :, ts(0, 128)])
        nc.sync.dma_start(out=wk_sb[:, :, ts(0, 128)], in_=wk_r[:, :, ts(0, 128)])
        x_sb = wpool.tile([128, KO, T], BF)
        for kc in range(KO):
            nc.sync.dma_start(
                out=x_sb[:, kc, 0 : T // 2],
                in_=xT.ap()[ds(128 * kc, 128), 0 : T // 2],
            )
        for fc in range(1, NFC):
            nc.sync.dma_start(
                out=wq_sb[:, :, ts(fc, 128)], in_=wq_r[:, :, ts(fc, 128)]
            )
            nc.sync.dma_start(
                out=wk_sb[:, :, ts(fc, 128)], in_=wk_r[:, :, ts(fc, 128)]
            )
        # broadcast biases across partitions (for token-major layouts)
        bv_bc = wpool.tile([128, F], FP32)
        nc.sync.dma_start(
            out=bv_bc,
            in_=bass.AP(tensor=bv.ap().tensor, offset=0, ap=[[0, 128], [1, F]]),
        )
        wv_sb = wpool.tile([128, KO, F], BF)
        nc.sync.dma_start(out=wv_sb, in_=wv.rearrange("(ko p) f -> p ko f", p=128))
        xT_r = xT.rearrange("(ko p) t -> p ko t", p=128)
        nc.sync.dma_start(out=x_sb[:, :, T // 2 :], in_=xT_r[:, :, T // 2 :])
        wp_sb = wpool.tile([128, NFC, C], BF)
        nc.sync.dma_start(out=wp_sb, in_=wp.rearrange("(ko p) n -> p ko n", p=128))
        bp_bc = wpool.tile([128, C], FP32)
        nc.sync.dma_start(
            out=bp_bc,
            in_=bass.AP(tensor=bp.ap().tensor, offset=0, ap=[[0, 128], [1, C]]),
        )

        # ---- persistent activations ----
        qT_sb = apool.tile([128, NFC, T], BF)   # q, feature-major
        kT_sb = apool.tile([128, NFC, T], BF)   # k, feature-major
        # v token-major, 66-stride per head: cols 0:64 = v, col 64 = ones
        v_sb = apool.tile([128, NKC, H_LOC, 66], BF)
        nc.vector.memset(v_sb[:, :, :, 64:65], 1.0)
        yT_sb = apool.tile([128, NFC, T], BF)   # attention out, feature-major

        partial = dpool.tile([T, C], BF)        # c_proj partial (pre-reduce)
        # per-Q-block ReduceScatter halves: core keeps [256,1024] per block
        rs_outs = [dpool.tile([256, C], BF, name=f"rs_out{q}") for q in range(NQ)]

        # ---- qkv projection units (emitted piecemeal: half up front, the
        # rest interleaved into the exp-bound attention phase as PE filler) --
        def qk_unit(w_sb, b_sb, dst, fc, tq2):
            # one 1024-token span of q^T or k^T for head-pair chunk fc
            ps = ps_s.tile([128, 1024], FP32, tag="sAB")
            for kc in range(KO):
                for half in range(2):
                    nc.tensor.matmul(
                        ps[:, ts(half, 512)],
                        lhsT=w_sb[:, kc, ts(fc, 128)],
                        rhs=x_sb[:, kc, ds(tq2 * 1024 + half * 512, 512)],
                        start=(kc == 0),
                        stop=(kc == KO - 1),
                    )
            nc.scalar.activation(
                out=dst[:, fc, ts(tq2, 1024)],
                in_=ps,
                func=mybir.ActivationFunctionType.Identity,
                bias=b_sb[:, fc : fc + 1],
                scale=1.0,
            )

        def v_unit(tc_i):
            ps = ps_s.tile([128, 1024], FP32, tag="sAB")
            for kc in range(KO):
                nc.tensor.matmul(
                    ps[:, 0:512],
                    lhsT=x_sb[:, kc, ts(tc_i, 128)],
                    rhs=wv_sb[:, kc, :],
                    start=(kc == 0),
                    stop=(kc == KO - 1),
                )
            nc.vector.tensor_add(
                out=v_sb[:, tc_i, :, 0:64],
                in0=ps[:, 0:512].rearrange("p (h f) -> p h f", h=H_LOC),
                in1=bv_bc.rearrange("p (h f) -> p h f", h=H_LOC),
            )

        # prefix: everything attention blocks 0-1 need
        for fc in range(NFC):
            qk_unit(wq_sb, bq_sb, qT_sb, fc, 0)
            qk_unit(wk_sb, bk_sb, kT_sb, fc, 0)
        for tc_i in range(8):
            v_unit(tc_i)

        def proj_tb(Q, tb):
            # bf16 partial (+0.5*bp, host-folded) for token row-block Q*4+tb
            trow = Q * 4 + tb
            ps = ps_s.tile([128, 1024], FP32, tag="sAB")
            for ncol in range(NCOL):
                for fc in range(NFC):
                    nc.tensor.matmul(
                        ps[:, ts(ncol, 512)],
                        lhsT=yT_sb[:, fc, ts(trow, 128)],
                        rhs=wp_sb[:, fc, ts(ncol, 512)],
                        start=(fc == 0),
                        stop=(fc == NFC - 1),
                    )
            p_sb = outp.tile([128, 1024], BF, tag="psb")
            nc.vector.tensor_add(out=p_sb, in0=ps, in1=bp_bc)
            nc.sync.dma_start(out=partial[ds(trow * 128, 128), :], in_=p_sb)

        def rs_block(Q):
            # reduce this 512-token block across the batch pair while later
            # blocks still compute; each core keeps 256 of the 512 rows.
            nc.gpsimd.collective_compute(
                "ReduceScatter",
                mybir.AluOpType.add,
                replica_groups=REPLICA_GROUPS,
                ins=[partial[ds(Q * 512, 512), :]],
                outs=[rs_outs[Q][:]],
            )

        def copy_out(Q):
            # bf16 shard -> output rows; emitted well after rs_block(Q) so
            # the sync-queue sem wait is (nearly) clear when it issues.
            nc.sync.dma_start(out=out.ap()[ds(Q * 256, 256), :], in_=rs_outs[Q][:])

        # filler units with emission deadlines:
        #   qT second halves + v 8..11 -> before attention(2) reads them
        #   kT second halves + v 12..15 -> before attention(3)
        # proj(Q) rides block Q+1's slack; rs(Q) fires as soon as the gpsimd
        # queue drains past the preceding normalize broadcasts.
        slot_work = {
            0: {fc: [lambda fc=fc: qk_unit(wq_sb, bq_sb, qT_sb, fc, 1),
                     lambda i=fc + 8: v_unit(i)] for fc in range(NFC)},
            1: {fc: [lambda fc=fc: qk_unit(wk_sb, bk_sb, kT_sb, fc, 1),
                     lambda i=fc + 12: v_unit(i),
                     lambda tb=fc: proj_tb(0, tb)] for fc in range(NFC)},
            2: {0: [lambda: proj_tb(1, 0)],
                1: [lambda: proj_tb(1, 1)],
                2: [lambda: proj_tb(1, 2)],
                3: [lambda: proj_tb(1, 3)]},
            3: {0: [lambda: proj_tb(2, 0), lambda: proj_tb(2, 1)],
                1: [lambda: proj_tb(2, 2), lambda: proj_tb(2, 3)],
                2: [lambda: rs_block(2)]},
        }
        after_block = {
            1: [lambda: rs_block(0)],
            2: [lambda: rs_block(1), lambda: copy_out(0)],
        }

        # ---- attention per q-block ----
        def attention_block(Q, slots):
            nkc = 4 * Q + 4  # causal: only key chunks 0 .. 4Q+3 contribute
            LAG = 3  # AV matmuls trail the QK/exp pipeline by this many chunks
            for fc in range(NFC):  # head pair (2fc, 2fc+1)
                oA = ps_o.tile([65, 512], FP32, tag="oA")
                oB = ps_o.tile([65, 512], FP32, tag="oB")
                pbuf = {}

                def emit_av(kc, oA=oA, oB=oB, nkc=nkc, fc=fc, Q=Q):
                    pAB = pbuf.pop(kc)
                    q0 = max(0, 128 * (kc - 4 * Q))
                    w = 512 - q0
                    nc.tensor.matmul(
                        oA[:, ds(q0, w)],
                        lhsT=v_sb[:, kc, 2 * fc, 0:65],
                        rhs=pAB[:, ds(q0, w)],
                        start=(kc == 0),
                        stop=(kc == nkc - 1),
                    )
                    nc.tensor.matmul(
                        oB[:, ds(q0, w)],
                        lhsT=v_sb[:, kc, 2 * fc + 1, 0:65],
                        rhs=pAB[:, ds(512 + q0, w)],
                        start=(kc == 0),
                        stop=(kc == nkc - 1),
                    )

                for kc in range(nkc):
                    # heads A and B share one 2-bank psum tile: A in cols
                    # 0:512 (array rows 0:64), B in 512:1024 (rows 64:128);
                    # the row-tiled pair runs concurrently on the PE.
                    # Diagonal chunks only touch query columns >= q0.
                    q0 = max(0, 128 * (kc - 4 * Q))
                    w = 512 - q0
                    sAB = ps_s.tile([128, 1024], FP32, tag="sAB")
                    nc.tensor.matmul(
                        sAB[:, ds(q0, w)],
                        lhsT=kT_sb[0:64, fc, ts(kc, 128)],
                        rhs=qT_sb[0:64, fc, ds(512 * Q + q0, w)],
                        start=True,
                        stop=True,
                        tile_position=(0, 0),
                    )
                    nc.tensor.matmul(
                        sAB[:, ds(512 + q0, w)],
                        lhsT=kT_sb[64:128, fc, ts(kc, 128)],
                        rhs=qT_sb[64:128, fc, ds(512 * Q + q0, w)],
                        start=True,
                        stop=True,
                        tile_position=(64, 0),
                    )
                    pAB = ppool.tile([128, 1024], BF, tag="pAB", bufs=5)
                    nc.scalar.activation(
                        out=pAB.rearrange("p (h q) -> p h q", h=2)[:, :, q0:],
                        in_=sAB.rearrange("p (h q) -> p h q", h=2)[:, :, q0:],
                        func=mybir.ActivationFunctionType.Exp,
                        scale=0.125,
                    )
                    if kc >= 4 * Q:
                        # crosses the causal boundary: zero exp of masked
                        # scores (k_global > q_global) for both head halves
                        nc.gpsimd.affine_select(
                            out=pAB.rearrange("p (h q) -> p h q", h=2)[:, :, q0:],
                            in_=pAB.rearrange("p (h q) -> p h q", h=2)[:, :, q0:],
                            compare_op=mybir.AluOpType.is_ge,
                            fill=0.0,
                            base=0,
                            channel_multiplier=-1,
                            pattern=[[0, 2], [1, w]],
                        )
                    pbuf[kc] = pAB
                    if kc >= LAG:
                        emit_av(kc - LAG)
                for kc in range(max(0, nkc - LAG), nkc):
                    emit_av(kc)
                # normalize: yT_h = oT[0:64] * (1 / oT[64]).
                oA_sb = npool.tile([65, 512], FP32, tag="oAsb")
                oB_sb = npool.tile([65, 512], FP32, tag="oBsb")
                nc.vector.tensor_copy(out=oA_sb, in_=oA)
                nc.vector.tensor_copy(out=oB_sb, in_=oB)
                # custom-DVE reciprocal_approx_fast mishandles inputs at a
                # nonzero partition base -- stage row 64 down to partition 0
                rzA = npool.tile([1, 512], FP32, tag="rzA", bufs=1)
                rzB = npool.tile([1, 512], FP32, tag="rzB", bufs=1)
                nc.vector.tensor_copy(out=rzA, in_=oA_sb[64:65, :])
                nc.vector.tensor_copy(out=rzB, in_=oB_sb[64:65, :])
                rA = npool.tile([1, 512], FP32, tag="rA", bufs=1)
                rB = npool.tile([1, 512], FP32, tag="rB", bufs=1)
                nc.vector.reciprocal_approx_fast(out=rA, in_=rzA)
                nc.vector.reciprocal_approx_fast(out=rB, in_=rzB)
                # partition-broadcast on GpSimd (no DRAM bounce)
                bcA = npool.tile([64, 512], FP32, tag="bcA", bufs=1)
                bcB = npool.tile([64, 512], FP32, tag="bcB", bufs=1)
                nc.gpsimd.partition_broadcast(bcA, rA)
                nc.gpsimd.partition_broadcast(bcB, rB)
                # head A lives on partitions 0:64 of chunk fc
                nc.vector.tensor_mul(
                    out=yT_sb[0:64, fc, ts(Q, 512)], in0=oA_sb[0:64, :], in1=bcA
                )
                # head B must land on partitions 64:128 -> stage + DMA shift
                yB = npool.tile([64, 512], BF, tag="yB")
                nc.vector.tensor_mul(out=yB, in0=oB_sb[0:64, :], in1=bcB)
                nc.sync.dma_start(out=yT_sb[64:128, fc, ts(Q, 512)], in_=yB)

                for work in slots.get(fc, ()):
                    work()

        for Q in range(NQ):
            attention_block(Q, slot_work.get(Q, {}))
            for work in after_block.get(Q, ()):
                work()
        # tail: only the last block's proj + reduce remain
        for tb in range(4):
            proj_tb(NQ - 1, tb)
        copy_out(1)
        rs_block(NQ - 1)
        copy_out(2)
        copy_out(NQ - 1)


_NC_CACHE = None


def _get_nc():
    global _NC_CACHE
    if _NC_CACHE is None:
        _NC_CACHE = _build_nc()
    return _NC_CACHE


def kernel(x, w_attn, b_attn, w_proj, b_proj):
    x = np.asarray(x)
    w_attn = np.asarray(w_attn)
    b_attn = np.asarray(b_attn)
    w_proj = np.asarray(w_proj)
    b_proj = np.asarray(b_proj)

    nc = _get_nc()

    in_maps = []
    for i in range(N_CORES):
        b, g = i // 2, i % 2
        in_maps.append(
            {
                "xT": np.ascontiguousarray(x[b].T).astype(BF16),
                "wq": np.ascontiguousarray(w_attn[:, g * F : (g + 1) * F]).astype(BF16),
                "wk": np.ascontiguousarray(
                    w_attn[:, C + g * F : C + (g + 1) * F]
                ).astype(BF16),
                "wv": np.ascontiguousarray(
                    w_attn[:, 2 * C + g * F : 2 * C + (g + 1) * F]
                ).astype(BF16),
                "bq": np.ascontiguousarray(b_attn[g * F : (g + 1) * F]).astype(
                    np.float32
                ),
                "bk": np.ascontiguousarray(b_attn[C + g * F : C + (g + 1) * F]).astype(
                    np.float32
                ),
                "bv": np.ascontiguousarray(
                    b_attn[2 * C + g * F : 2 * C + (g + 1) * F]
                ).astype(np.float32),
                "wp": np.ascontiguousarray(w_proj[g * F : (g + 1) * F, :]).astype(BF16),
                "bp": (b_proj * 0.5).astype(np.float32),
            }
        )

    global _last_in_maps
    _last_in_maps = in_maps  # stashed for external profiling harnesses
    res = run_bass_kernel_spmd(nc, in_maps, core_ids=list(range(N_CORES)))

    # Each core's "out" holds NQ blocks of 256 rows: block Q is the core's
    # ReduceScatter half of token rows [Q*512, (Q+1)*512) -- rank 0 (even
    # core) the first 256, rank 1 (odd core) the last 256.
    out = np.empty((B, T, C), dtype=np.float32)
    for b in range(B):
        even = res.results[2 * b]["out"].astype(np.float32).reshape(NQ, 256, C)
        odd = res.results[2 * b + 1]["out"].astype(np.float32).reshape(NQ, 256, C)
        blocks = out[b].reshape(NQ, 2, 256, C)
        blocks[:, 0] = even
        blocks[:, 1] = odd
    return out


# revision 40
# speedup vs baseline: 1.0488x; 1.0483x over previous
"""Causal self-attention (B=4, T=2048, C=1024, NH=16) on 8 TRN2 NeuronCores.

Sharding (per spec hint): tensor-parallel over heads x data-parallel over batch.
Core i handles batch b = i//2 and head-group g = i%2 (8 heads each).
  - c_attn column-parallel: each core computes q,k,v for its 8 heads.
  - attention: fully local per core (its heads, its batch element).
  - c_proj row-parallel: each core computes a bf16 partial (yT@wp + 0.5*bp)
    per 512-token block; a 2-core ReduceScatter over pairs
    [[0,1],[2,3],[4,5],[6,7]] sums each block's partials as soon as they
    exist; cheap bf16 DRAM copies move each 256-row shard to the output,
    overlapped except for the final block. Host casts to fp32 + concatenates.

Device algorithm (per core), all matmuls bf16 with fp32 PSUM accumulation:
  xT (C,T) staged transposed by host; input DMAs are split per weight/x chunk
  and ordered by first use, so the first qkv matmul starts after ~0.8MB
  instead of the full 8MB.
  qT = wq^T @ xT, kT = wk^T @ xT   (feature-major, 4 chunks of 128)
  v  = x @ wv                      (token-major) + ones column per head
  per head pair (2fc, 2fc+1), per q-block Q (512 wide):
    s^T[kchunk] = kT_h^T @ qT_h    (K=64 contraction, row-tiled pair -> concurrent)
    p = exp(0.125 * s^T)  (ScalarE, bf16 out); causal-zeroed on GpSimd for
        diagonal chunks; fully-masked chunks skipped entirely. Diagonal
        chunks are triangle-trimmed: QK/exp/select/AV only touch query
        columns >= 128*(kc-4Q), cutting ~12% of PE work and ~15% of exp.
    o^T[65,512] += v_aug_h^T @ p   (v_aug has a ones column -> row 64 = softmax
        denominators, fused into the same matmul)
    yT_h = o^T[0:64] * (1/o^T[64])  (DVE recip + GpSimd partition_broadcast,
        no DRAM bounce)
  c_proj token-blocks ride the exp-bound slack of later attention blocks;
  the tail is only: proj(block 3) -> RS(3) -> done.
"""

import sys

if "/opt/trn_rl_repo" not in sys.path:
    sys.path.insert(0, "/opt/trn_rl_repo")

import numpy as np
import ml_dtypes

import concourse.bass as bass
import concourse.bacc as bacc
import concourse.mybir as mybir
import concourse.tile as tile
from concourse.bass import ts, ds
from concourse.bass_utils import run_bass_kernel_spmd

BF16 = ml_dtypes.bfloat16
N_CORES = 8
B, T, C = 4, 2048, 1024
NH, HS = 16, 64
H_LOC = NH // 2        # heads per core
F = H_LOC * HS         # 512 local qkv features
NFC = F // 128         # 4 feature chunks (one head pair each)
NKC = T // 128         # 16 key chunks
NQ = T // 512          # 4 query blocks
NCOL = C // 512        # 2 output column blocks
KO = C // 128          # 8 contraction chunks for the qkv projections
REPLICA_GROUPS = [[0, 1], [2, 3], [4, 5], [6, 7]]

FP32 = mybir.dt.float32
BF = mybir.dt.bfloat16


def _build_nc():
    # Bacc (not plain Bass): its compile() pipeline runs
    # generate_event_semaphores, which splits sync waits so no instruction
    # carries more than the hardware allows (walrus rejects >1 otherwise).
    nc = bacc.Bacc(None, target_bir_lowering=False, num_devices=N_CORES)

    xT = nc.dram_tensor("xT", [C, T], BF, kind="ExternalInput")
    wq = nc.dram_tensor("wq", [C, F], BF, kind="ExternalInput")
    wk = nc.dram_tensor("wk", [C, F], BF, kind="ExternalInput")
    wv = nc.dram_tensor("wv", [C, F], BF, kind="ExternalInput")
    bq = nc.dram_tensor("bq", [F], FP32, kind="ExternalInput")
    bk = nc.dram_tensor("bk", [F], FP32, kind="ExternalInput")
    bv = nc.dram_tensor("bv", [F], FP32, kind="ExternalInput")
    wp = nc.dram_tensor("wp", [F, C], BF, kind="ExternalInput")
    bp = nc.dram_tensor("bp", [C], FP32, kind="ExternalInput")
    out = nc.dram_tensor("out", [T // 2, C], BF, kind="ExternalOutput")

    with tile.TileContext(nc) as tc:
        _body(tc, xT, wq, wk, wv, bq, bk, bv, wp, bp, out)
    nc.compile()
    return nc


def _body(tc, xT, wq, wk, wv, bq, bk, bv, wp, bp, out):
    nc = tc.nc
    import contextlib

    ctx = contextlib.ExitStack()
    with ctx:
        wpool = ctx.enter_context(tc.tile_pool(name="weights", bufs=1))
        apool = ctx.enter_context(tc.tile_pool(name="acts", bufs=1))
        ppool = ctx.enter_context(tc.tile_pool(name="ptiles", bufs=3))
        npool = ctx.enter_context(tc.tile_pool(name="norm", bufs=2))
        outp = ctx.enter_context(tc.tile_pool(name="outsb", bufs=2))
        # PSUM budget (8 banks): sAB [128,1024] x3 bufs = 6, oA/oB 1 bank each = 2
        ps_s = ctx.enter_context(tc.tile_pool(name="ps_s", bufs=3, space="PSUM"))
        ps_o = ctx.enter_context(tc.tile_pool(name="ps_o", bufs=1, space="PSUM"))
        dpool = ctx.enter_context(tc.tile_pool(name="dram", bufs=1, space="DRAM"))

        # ---- stage inputs into SBUF, split per chunk and ordered by first
        # use: the prefix consumes (wq|wk chunk fc=0) + all 8 x chunks of
        # tokens 0:1024 first, so matmuls start after ~0.8MB of DMA.
        wq_sb = wpool.tile([128, KO, F], BF)
        wk_sb = wpool.tile([128, KO, F], BF)
        wq_r = wq.rearrange("(ko p) f -> p ko f", p=128)
        wk_r = wk.rearrange("(ko p) f -> p ko f", p=128)
        nc.sync.dma_start(out=wq_sb[:, :, ts(0, 128)], in_=wq_r[:, :, ts(0, 128)])
        nc.sync.dma_start(out=wk_sb[:, :, ts(0, 128)], in_=wk_r[:, :, ts(0, 128)])
        x_sb = wpool.tile([128, KO, T], BF)
        for kc in range(KO):
            nc.sync.dma_start(
                out=x_sb[:, kc, 0 : T // 2],
                in_=xT.ap()[ds(128 * kc, 128), 0 : T // 2],
            )
        # tiny partition-strided bias DMAs (128 descriptors each) ride behind
        # the first x chunks; first use is the end of the fc=0 unit anyway
        bq_sb = wpool.tile([128, NFC], FP32)
        nc.sync.dma_start(out=bq_sb, in_=bq.rearrange("(fo p) -> p fo", p=128))
        bk_sb = wpool.tile([128, NFC], FP32)
        nc.sync.dma_start(out=bk_sb, in_=bk.rearrange("(fo p) -> p fo", p=128))
        for fc in range(1, NFC):
            nc.sync.dma_start(
                out=wq_sb[:, :, ts(fc, 128)], in_=wq_r[:, :, ts(fc, 128)]
            )
            nc.sync.dma_start(
                out=wk_sb[:, :, ts(fc, 128)], in_=wk_r[:, :, ts(fc, 128)]
            )
        # broadcast biases across partitions (for token-major layouts)
        bv_bc = wpool.tile([128, F], FP32)
        nc.sync.dma_start(
            out=bv_bc,
            in_=bass.AP(tensor=bv.ap().tensor, offset=0, ap=[[0, 128], [1, F]]),
        )
        wv_sb = wpool.tile([128, KO, F], BF)
        nc.sync.dma_start(out=wv_sb, in_=wv.rearrange("(ko p) f -> p ko f", p=128))
        xT_r = xT.rearrange("(ko p) t -> p ko t", p=128)
        nc.sync.dma_start(out=x_sb[:, :, T // 2 :], in_=xT_r[:, :, T // 2 :])
        wp_sb = wpool.tile([128, NFC, C], BF)
        nc.sync.dma_start(out=wp_sb, in_=wp.rearrange("(ko p) n -> p ko n", p=128))
        bp_bc = wpool.tile([128, C], FP32)
        nc.sync.dma_start(
            out=bp_bc,
            in_=bass.AP(tensor=bp.ap().tensor, offset=0, ap=[[0, 128], [1, C]]),
        )

        # ---- persistent activations ----
        qT_sb = apool.tile([128, NFC, T], BF)   # q, feature-major
        kT_sb = apool.tile([128, NFC, T], BF)   # k, feature-major
        # v token-major, 66-stride per head: cols 0:64 = v, col 64 = ones
        v_sb = apool.tile([128, NKC, H_LOC, 66], BF)
        nc.vector.memset(v_sb[:, :, :, 64:65], 1.0)
        yT_sb = apool.tile([128, NFC, T], BF)   # attention out, feature-major

        partial = dpool.tile([T, C], BF)        # c_proj partial (pre-reduce)
        # per-Q-block ReduceScatter halves: core keeps [256,1024] per block
        rs_outs = [dpool.tile([256, C], BF, name=f"rs_out{q}") for q in range(NQ)]

        # ---- qkv projection units (emitted piecemeal: half up front, the
        # rest interleaved into the exp-bound attention phase as PE filler) --
        def qk_unit(w_sb, b_sb, dst, fc, tq2):
            # one 1024-token span of q^T or k^T for head-pair chunk fc
            ps = ps_s.tile([128, 1024], FP32, tag="sAB")
            for kc in range(KO):
                for half in range(2):
                    nc.tensor.matmul(
                        ps[:, ts(half, 512)],
                        lhsT=w_sb[:, kc, ts(fc, 128)],
                        rhs=x_sb[:, kc, ds(tq2 * 1024 + half * 512, 512)],
                        start=(kc == 0),
                        stop=(kc == KO - 1),
                    )
            nc.scalar.activation(
                out=dst[:, fc, ts(tq2, 1024)],
                in_=ps,
                func=mybir.ActivationFunctionType.Identity,
                bias=b_sb[:, fc : fc + 1],
                scale=1.0,
            )

        def v_unit(tc_i):
            ps = ps_s.tile([128, 1024], FP32, tag="sAB")
            for kc in range(KO):
                nc.tensor.matmul(
                    ps[:, 0:512],
                    lhsT=x_sb[:, kc, ts(tc_i, 128)],
                    rhs=wv_sb[:, kc, :],
                    start=(kc == 0),
                    stop=(kc == KO - 1),
                )
            nc.vector.tensor_add(
                out=v_sb[:, tc_i, :, 0:64],
                in0=ps[:, 0:512].rearrange("p (h f) -> p h f", h=H_LOC),
                in1=bv_bc.rearrange("p (h f) -> p h f", h=H_LOC),
            )

        # prefix: everything attention blocks 0-1 need
        for fc in range(NFC):
            qk_unit(wq_sb, bq_sb, qT_sb, fc, 0)
            qk_unit(wk_sb, bk_sb, kT_sb, fc, 0)
        for tc_i in range(8):
            v_unit(tc_i)

        def proj_tb(Q, tb):
            # bf16 partial (+0.5*bp, host-folded) for token row-block Q*4+tb
            trow = Q * 4 + tb
            ps = ps_s.tile([128, 1024], FP32, tag="sAB")
            # fc outer / ncol inner: consecutive matmuls alternate the two
            # PSUM banks of ps (same per-element accumulation order, but
            # back-to-back issue avoids the same-bank write penalty)
            for fc in range(NFC):
                for ncol in range(NCOL):
                    nc.tensor.matmul(
                        ps[:, ts(ncol, 512)],
                        lhsT=yT_sb[:, fc, ts(trow, 128)],
                        rhs=wp_sb[:, fc, ts(ncol, 512)],
                        start=(fc == 0),
                        stop=(fc == NFC - 1),
                    )
            p_sb = outp.tile([128, 1024], BF, tag="psb")
            nc.vector.tensor_add(out=p_sb, in0=ps, in1=bp_bc)
            nc.sync.dma_start(out=partial[ds(trow * 128, 128), :], in_=p_sb)

        def rs_block(Q):
            # reduce this 512-token block across the batch pair while later
            # blocks still compute; each core keeps 256 of the 512 rows.
            nc.gpsimd.collective_compute(
                "ReduceScatter",
                mybir.AluOpType.add,
                replica_groups=REPLICA_GROUPS,
                ins=[partial[ds(Q * 512, 512), :]],
                outs=[rs_outs[Q][:]],
            )

        def copy_out(Q):
            # bf16 shard -> output rows; emitted well after rs_block(Q) so
            # the sync-queue sem wait is (nearly) clear when it issues.
            nc.sync.dma_start(out=out.ap()[ds(Q * 256, 256), :], in_=rs_outs[Q][:])

        # filler units with emission deadlines:
        #   qT second halves + v 8..11 -> before attention(2) reads them
        #   kT second halves + v 12..15 -> before attention(3)
        # proj(Q) rides block Q+1's slack; rs(Q) fires as soon as the gpsimd
        # queue drains past the preceding normalize broadcasts.
        slot_work = {
            0: {fc: [lambda fc=fc: qk_unit(wq_sb, bq_sb, qT_sb, fc, 1),
                     lambda i=fc + 8: v_unit(i)] for fc in range(NFC)},
            1: {fc: [lambda fc=fc: qk_unit(wk_sb, bk_sb, kT_sb, fc, 1),
                     lambda i=fc + 12: v_unit(i),
                     lambda tb=fc: proj_tb(0, tb)] for fc in range(NFC)},
            2: {0: [lambda: proj_tb(1, 0)],
                1: [lambda: proj_tb(1, 1)],
                2: [lambda: proj_tb(1, 2)],
                3: [lambda: proj_tb(1, 3)]},
            3: {0: [lambda: proj_tb(2, 0), lambda: proj_tb(2, 1)],
                1: [lambda: proj_tb(2, 2), lambda: proj_tb(2, 3)],
                2: [lambda: rs_block(2)]},
        }
        after_block = {
            1: [lambda: rs_block(0)],
            2: [lambda: rs_block(1), lambda: copy_out(0)],
        }

        # ---- attention per q-block ----
        def attention_block(Q, slots):
            nkc = 4 * Q + 4  # causal: only key chunks 0 .. 4Q+3 contribute
            LAG = 3  # AV matmuls trail the QK/exp pipeline by this many chunks
            for fc in range(NFC):  # head pair (2fc, 2fc+1)
                oA = ps_o.tile([65, 512], FP32, tag="oA")
                oB = ps_o.tile([65, 512], FP32, tag="oB")
                pbuf = {}

                def emit_av(kc, oA=oA, oB=oB, nkc=nkc, fc=fc, Q=Q):
                    pAB = pbuf.pop(kc)
                    q0 = max(0, 128 * (kc - 4 * Q))
                    w = 512 - q0
                    nc.tensor.matmul(
                        oA[:, ds(q0, w)],
                        lhsT=v_sb[:, kc, 2 * fc, 0:65],
                        rhs=pAB[:, ds(q0, w)],
                        start=(kc == 0),
                        stop=(kc == nkc - 1),
                    )
                    nc.tensor.matmul(
                        oB[:, ds(q0, w)],
                        lhsT=v_sb[:, kc, 2 * fc + 1, 0:65],
                        rhs=pAB[:, ds(512 + q0, w)],
                        start=(kc == 0),
                        stop=(kc == nkc - 1),
                    )

                for kc in range(nkc):
                    # heads A and B share one 2-bank psum tile: A in cols
                    # 0:512 (array rows 0:64), B in 512:1024 (rows 64:128);
                    # the row-tiled pair runs concurrently on the PE.
                    # Diagonal chunks only touch query columns >= q0.
                    q0 = max(0, 128 * (kc - 4 * Q))
                    w = 512 - q0
                    sAB = ps_s.tile([128, 1024], FP32, tag="sAB")
                    nc.tensor.matmul(
                        sAB[:, ds(q0, w)],
                        lhsT=kT_sb[0:64, fc, ts(kc, 128)],
                        rhs=qT_sb[0:64, fc, ds(512 * Q + q0, w)],
                        start=True,
                        stop=True,
                        tile_position=(0, 0),
                    )
                    nc.tensor.matmul(
                        sAB[:, ds(512 + q0, w)],
                        lhsT=kT_sb[64:128, fc, ts(kc, 128)],
                        rhs=qT_sb[64:128, fc, ds(512 * Q + q0, w)],
                        start=True,
                        stop=True,
                        tile_position=(64, 0),
                    )
                    pAB = ppool.tile([128, 1024], BF, tag="pAB", bufs=5)
                    nc.scalar.activation(
                        out=pAB.rearrange("p (h q) -> p h q", h=2)[:, :, q0:],
                        in_=sAB.rearrange("p (h q) -> p h q", h=2)[:, :, q0:],
                        func=mybir.ActivationFunctionType.Exp,
                        scale=0.125,
                    )
                    if kc >= 4 * Q:
                        # crosses the causal boundary: zero exp of masked
                        # scores (k_global > q_global) for both head halves
                        nc.gpsimd.affine_select(
                            out=pAB.rearrange("p (h q) -> p h q", h=2)[:, :, q0:],
                            in_=pAB.rearrange("p (h q) -> p h q", h=2)[:, :, q0:],
                            compare_op=mybir.AluOpType.is_ge,
                            fill=0.0,
                            base=0,
                            channel_multiplier=-1,
                            pattern=[[0, 2], [1, w]],
                        )
                    pbuf[kc] = pAB
                    if kc >= LAG:
                        emit_av(kc - LAG)
                for kc in range(max(0, nkc - LAG), nkc):
                    emit_av(kc)
                # normalize: yT_h = oT[0:64] * (1 / oT[64]).
                oA_sb = npool.tile([65, 512], FP32, tag="oAsb")
                oB_sb = npool.tile([65, 512], FP32, tag="oBsb")
                nc.vector.tensor_copy(out=oA_sb, in_=oA)
                nc.vector.tensor_copy(out=oB_sb, in_=oB)
                # custom-DVE reciprocal_approx_fast mishandles inputs at a
                # nonzero partition base -- stage row 64 down to partition 0
                rzA = npool.tile([1, 512], FP32, tag="rzA", bufs=1)
                rzB = npool.tile([1, 512], FP32, tag="rzB", bufs=1)
                nc.vector.tensor_copy(out=rzA, in_=oA_sb[64:65, :])
                nc.vector.tensor_copy(out=rzB, in_=oB_sb[64:65, :])
                rA = npool.tile([1, 512], FP32, tag="rA", bufs=1)
                rB = npool.tile([1, 512], FP32, tag="rB", bufs=1)
                nc.vector.reciprocal_approx_fast(out=rA, in_=rzA)
                nc.vector.reciprocal_approx_fast(out=rB, in_=rzB)
                # partition-broadcast on GpSimd (no DRAM bounce)
                bcA = npool.tile([64, 512], FP32, tag="bcA", bufs=1)
                bcB = npool.tile([64, 512], FP32, tag="bcB", bufs=1)
                nc.gpsimd.partition_broadcast(bcA, rA)
                nc.gpsimd.partition_broadcast(bcB, rB)
                # head A lives on partitions 0:64 of chunk fc
                nc.vector.tensor_mul(
                    out=yT_sb[0:64, fc, ts(Q, 512)], in0=oA_sb[0:64, :], in1=bcA
                )
                # head B must land on partitions 64:128 -> stage + DMA shift
                yB = npool.tile([64, 512], BF, tag="yB")
                nc.vector.tensor_mul(out=yB, in0=oB_sb[0:64, :], in1=bcB)
                nc.sync.dma_start(out=yT_sb[64:128, fc, ts(Q, 512)], in_=yB)

                for work in slots.get(fc, ()):
                    work()

        for Q in range(NQ):
            attention_block(Q, slot_work.get(Q, {}))
            for work in after_block.get(Q, ()):
                work()
        # tail: only the last block's proj + reduce remain
        for tb in range(4):
            proj_tb(NQ - 1, tb)
        copy_out(1)
        rs_block(NQ - 1)
        copy_out(2)
        copy_out(NQ - 1)


_NC_CACHE = None


def _get_nc():
    global _NC_CACHE
    if _NC_CACHE is None:
        _NC_CACHE = _build_nc()
    return _NC_CACHE


def kernel(x, w_attn, b_attn, w_proj, b_proj):
    x = np.asarray(x)
    w_attn = np.asarray(w_attn)
    b_attn = np.asarray(b_attn)
    w_proj = np.asarray(w_proj)
    b_proj = np.asarray(b_proj)

    nc = _get_nc()

    in_maps = []
    for i in range(N_CORES):
        b, g = i // 2, i % 2
        in_maps.append(
            {
                "xT": np.ascontiguousarray(x[b].T).astype(BF16),
                "wq": np.ascontiguousarray(w_attn[:, g * F : (g + 1) * F]).astype(BF16),
                "wk": np.ascontiguousarray(
                    w_attn[:, C + g * F : C + (g + 1) * F]
                ).astype(BF16),
                "wv": np.ascontiguousarray(
                    w_attn[:, 2 * C + g * F : 2 * C + (g + 1) * F]
                ).astype(BF16),
                "bq": np.ascontiguousarray(b_attn[g * F : (g + 1) * F]).astype(
                    np.float32
                ),
                "bk": np.ascontiguousarray(b_attn[C + g * F : C + (g + 1) * F]).astype(
                    np.float32
                ),
                "bv": np.ascontiguousarray(
                    b_attn[2 * C + g * F : 2 * C + (g + 1) * F]
                ).astype(np.float32),
                "wp": np.ascontiguousarray(w_proj[g * F : (g + 1) * F, :]).astype(BF16),
                "bp": (b_proj * 0.5).astype(np.float32),
            }
        )

    global _last_in_maps
    _last_in_maps = in_maps  # stashed for external profiling harnesses
    res = run_bass_kernel_spmd(nc, in_maps, core_ids=list(range(N_CORES)))

    # Each core's "out" holds NQ blocks of 256 rows: block Q is the core's
    # ReduceScatter half of token rows [Q*512, (Q+1)*512) -- rank 0 (even
    # core) the first 256, rank 1 (odd core) the last 256.
    out = np.empty((B, T, C), dtype=np.float32)
    for b in range(B):
        even = res.results[2 * b]["out"].astype(np.float32).reshape(NQ, 256, C)
        odd = res.results[2 * b + 1]["out"].astype(np.float32).reshape(NQ, 256, C)
        blocks = out[b].reshape(NQ, 2, 256, C)
        blocks[:, 0] = even
        blocks[:, 1] = odd
    return out
